# revision 4
# baseline (speedup 1.0000x reference)
"""MXFP4-quantized linear kernel for Trainium2 (8 NeuronCores, SPMD).

Problem: out = quant_mxfp4(x) @ W.T + bias
  x [2, 4096, 4096] f32, W [11008, 4096] f32, bias [11008] f32 -> out [2, 4096, 11008] f32

Strategy (data-parallel over rows of x; hint allows this):
  - Host: flatten x to [8192, 4096], shard rows 8 ways; pre-transpose W to
    WT [4096, 11008] and cast bf16 (static weight preprocessing); bias f32.
  - Each core: quantize its x shard (dynamic per-32-block MXFP4) entirely
    on-chip, DMA-transpose quantized bf16 tiles to K-major layout, then a
    dense bf16 GEMM (fp32 PSUM accumulate) against streamed WT tiles,
    adding bias during PSUM drain. No collectives needed.

MXFP4 snap (branch-free, works in 2x space so grid = {0,1,2,3,4,6,8,12}):
  scale_h = fp16(amax/6)/2 exactly; w = x * (1/scale_h)
  high |w|>=2: Veltkamp split with 2^22+1 -> RNE to 2-bit significand
  low  |w|< 2: (w + 1.5*2^23) - 1.5*2^23 -> RNE to integer
  blend via copy_predicated on mask = Relu(8 - w^2); x_q = s * scale_h (bf16)
Ties (exact midpoints) round to-even vs reference's to-lower: measure-zero.
"""
import sys

try:
    import concourse  # noqa: F401
except ImportError:
    sys.path.insert(0, "/opt/trn_rl_repo")

import numpy as np
import ml_dtypes

import concourse.bacc as bacc
import concourse.mybir as mybir
from concourse import tile
from concourse.bass_utils import run_bass_kernel_spmd

F32, F16, BF16 = mybir.dt.float32, mybir.dt.float16, mybir.dt.bfloat16
ACT = mybir.ActivationFunctionType
ALU = mybir.AluOpType

CV = float(2**22 + 1)      # Veltkamp split constant -> 2-bit significand RNE
CR = float(1.5 * 2**23)    # integer RNE trick constant

N_CORES = 8
B, S, K, N = 2, 4096, 4096, 11008
M = B * S                  # 8192
MS = M // N_CORES          # 1024 rows per core
QC = 512                   # quant chunk width (along K)


def build_program(Ms=MS, Kd=K, Nd=N, wt_bufs=56):
    """Build the SPMD Bass program for one core (same program on all cores)."""
    nc = bacc.Bacc("TRN2", target_bir_lowering=False, debug=False)
    x = nc.dram_tensor("x", [Ms, Kd], F32, kind="ExternalInput")
    wt = nc.dram_tensor("wt", [Kd, Nd], BF16, kind="ExternalInput")
    bias = nc.dram_tensor("bias", [Nd], F32, kind="ExternalInput")
    out = nc.dram_tensor("out", [Ms, Nd], F32, kind="ExternalOutput")

    MT = Ms // 128          # m-tiles per core
    KT = Kd // 128          # k-tiles
    NB = QC // 32           # quant blocks per chunk
    QCH = Kd // QC          # quant chunks per m-tile

    # n-chunks of <=512
    nchunks = []
    n0 = 0
    while n0 < Nd:
        nw = min(512, Nd - n0)
        nchunks.append((n0, nw))
        n0 += nw

    with tile.TileContext(nc) as tc:
        with (
            tc.tile_pool(name="xqt", bufs=1) as xqt_pool,
            tc.tile_pool(name="xin", bufs=3) as xin_pool,
            tc.tile_pool(name="qtmp", bufs=2) as qtmp_pool,
            tc.tile_pool(name="qsmall", bufs=3) as qsmall_pool,
            tc.tile_pool(name="xq", bufs=2) as xq_pool,
            tc.tile_pool(name="wtp", bufs=wt_bufs) as wt_pool,
            tc.tile_pool(name="bnc", bufs=3) as bias_pool,
            tc.tile_pool(name="outp", bufs=8) as out_pool,
            tc.tile_pool(name="cst", bufs=1) as const_pool,
            tc.tile_pool(name="psum", bufs=8, space="PSUM") as psum_pool,
        ):
            bias8 = const_pool.tile([128, 1], F32, tag="bias8")
            nc.vector.memset(bias8[:], 8.0)

            # persistent K-major quantized activations: KT tiles [128, Ms] bf16
            xqT = [
                xqt_pool.tile([128, Ms], BF16, tag=f"xqt{k}", name=f"xqt{k}")
                for k in range(KT)
            ]

            # ---- Phase A: quantize x, m-tile by m-tile ----
            for mt in range(MT):
                xqn = xq_pool.tile([128, Kd], BF16, tag="xqn", name=f"xqn{mt}")
                for q in range(QCH):
                    k0 = q * QC
                    xin = xin_pool.tile([128, QC], F32, tag="xin", name=f"xin{mt}_{q}")
                    nc.sync.dma_start(out=xin[:], in_=x[mt * 128:(mt + 1) * 128, k0:k0 + QC])

                    amax = qsmall_pool.tile([128, NB], F32, tag="amax", name=f"amax{mt}_{q}")
                    nc.vector.tensor_reduce(
                        out=amax[:], in_=xin.rearrange("p (b c) -> p b c", c=32),
                        axis=mybir.AxisListType.X, op=ALU.max,
                        apply_absolute_value=True)
                    sc16 = qsmall_pool.tile([128, NB], F16, tag="sc16", name=f"sc16{mt}_{q}")
                    nc.vector.tensor_scalar(out=sc16[:], in0=amax[:],
                                            scalar1=float(1.0 / 6.0),
                                            scalar2=None, op0=ALU.mult)
                    sch = qsmall_pool.tile([128, NB], F32, tag="sch", name=f"sch{mt}_{q}")
                    nc.vector.tensor_scalar(out=sch[:], in0=sc16[:], scalar1=0.5,
                                            scalar2=None, op0=ALU.mult)
                    r2 = qsmall_pool.tile([128, NB], F32, tag="r2", name=f"r2{mt}_{q}")
                    nc.vector.reciprocal(out=r2[:], in_=sch[:])

                    w = qtmp_pool.tile([128, QC], F32, tag="w", name=f"w{mt}_{q}")
                    nc.vector.tensor_tensor(
                        out=w.rearrange("p (b c) -> p b c", c=32),
                        in0=xin.rearrange("p (b c) -> p b c", c=32),
                        in1=r2.unsqueeze(2).broadcast_to([128, NB, 32]),
                        op=ALU.mult)

                    # high path (ACT for c, DVE for d/s)
                    c = qtmp_pool.tile([128, QC], F32, tag="c", name=f"c{mt}_{q}")
                    nc.scalar.activation(out=c[:], in_=w[:], func=ACT.Copy, scale=CV)
                    d = qtmp_pool.tile([128, QC], F32, tag="d", name=f"d{mt}_{q}")
                    nc.vector.tensor_tensor(out=d[:], in0=c[:], in1=w[:], op=ALU.subtract)
                    s = qtmp_pool.tile([128, QC], F32, tag="s", name=f"s{mt}_{q}")
                    nc.vector.tensor_tensor(out=s[:], in0=c[:], in1=d[:], op=ALU.subtract)

                    # low path on ACT
                    u = qtmp_pool.tile([128, QC], F32, tag="u", name=f"u{mt}_{q}")
                    nc.scalar.activation(out=u[:], in_=w[:], func=ACT.Copy, bias=CR)
                    sL = qtmp_pool.tile([128, QC], F32, tag="sL", name=f"sL{mt}_{q}")
                    nc.scalar.activation(out=sL[:], in_=u[:], func=ACT.Copy, bias=-CR)

                    # mask on ACT: Relu(8 - w^2) nonzero iff |w| < sqrt(8)
                    m1 = qtmp_pool.tile([128, QC], F32, tag="m1", name=f"m1{mt}_{q}")
                    nc.scalar.activation(out=m1[:], in_=w[:], func=ACT.Square)
                    mask = qtmp_pool.tile([128, QC], mybir.dt.uint8, tag="mask",
                                          name=f"mask{mt}_{q}")
                    nc.scalar.activation(out=mask[:], in_=m1[:], func=ACT.Relu,
                                         scale=-1.0, bias=bias8[:])
                    nc.vector.copy_predicated(out=s[:], mask=mask[:], data=sL[:])

                    nc.vector.tensor_tensor(
                        out=xqn[:, k0:k0 + QC].rearrange("p (b c) -> p b c", c=32),
                        in0=s.rearrange("p (b c) -> p b c", c=32),
                        in1=sch.unsqueeze(2).broadcast_to([128, NB, 32]),
                        op=ALU.mult)

                # transpose to K-major via DMA xbar (off the PE/DVE path)
                for kt in range(KT):
                    nc.sync.dma_start_transpose(
                        xqT[kt][:, mt * 128:(mt + 1) * 128],
                        xqn[:, kt * 128:(kt + 1) * 128])

            # ---- Phase B: GEMM out[m, n] = sum_k xq[m, k] * WT[k, n] + bias ----
            for nci, (n0, nw) in enumerate(nchunks):
                wts = []
                for k in range(KT):
                    wtt = wt_pool.tile([128, nw], BF16, tag="wt", name=f"wt{nci}_{k}")
                    nc.sync.dma_start(out=wtt[:], in_=wt[k * 128:(k + 1) * 128, n0:n0 + nw])
                    wts.append(wtt)

                bnc = bias_pool.tile([128, nw], F32, tag="bnc", name=f"bnc{nci}")
                nc.sync.dma_start(
                    out=bnc[:],
                    in_=bias[n0:n0 + nw].unsqueeze(0).broadcast_to([128, nw]))

                # wave structure: first n-chunk uses single-mt waves so the PE
                # can start right after the first m-tile is quantized
                if nci == 0:
                    waves = [[mt] for mt in range(MT)]
                else:
                    waves = [list(range(g, min(g + 4, MT))) for g in range(0, MT, 4)]

                for wave in waves:
                    psums = [
                        psum_pool.tile([128, nw], F32, tag="ps", name=f"ps{nci}_{mt}")
                        for mt in wave
                    ]
                    for k in range(KT):
                        for j, mt in enumerate(wave):
                            nc.tensor.matmul(
                                out=psums[j][:],
                                lhsT=xqT[k][:, mt * 128:(mt + 1) * 128],
                                rhs=wts[k][:],
                                start=(k == 0), stop=(k == KT - 1))
                    for j, mt in enumerate(wave):
                        ot = out_pool.tile([128, nw], F32, tag="ot", name=f"ot{nci}_{mt}")
                        nc.vector.tensor_tensor(out=ot[:], in0=psums[j][:],
                                                in1=bnc[:], op=ALU.add)
                        nc.sync.dma_start(
                            out=out[mt * 128:(mt + 1) * 128, n0:n0 + nw], in_=ot[:])
    nc.compile()
    return nc


_CACHE = {}


def _get_program():
    if "nc" not in _CACHE:
        _CACHE["nc"] = build_program()
    return _CACHE["nc"]


def run(x, W, bias, trace=False):
    nc = _get_program()
    xf = np.ascontiguousarray(np.asarray(x, dtype=np.float32).reshape(M, K))
    WT16 = np.ascontiguousarray(
        np.asarray(W, dtype=np.float32).T.astype(ml_dtypes.bfloat16))
    b32 = np.ascontiguousarray(np.asarray(bias, dtype=np.float32))
    in_maps = [
        {"x": xf[c * MS:(c + 1) * MS], "wt": WT16, "bias": b32}
        for c in range(N_CORES)
    ]
    res = run_bass_kernel_spmd(nc, in_maps, list(range(N_CORES)), trace=trace)
    outs = [res.results[c]["out"] for c in range(N_CORES)]
    full = np.concatenate(outs, axis=0).reshape(B, S, N)
    return full, res


def kernel(x, W, bias):
    out, _ = run(x, W, bias, trace=False)
    return out
